# revision 25
# baseline (speedup 1.0000x reference)
"""Classical self-attention (head-summed scores) on 8 trn2 NeuronCores.

Math (per batch b):
    Q = x Wq; K = x Wk; V = x Wv          (W_qkv split columns 3x1024)
    S = Q K^T / 8   (full-E contraction: heads+dims summed)
    P = softmax(S, axis=-1)
    out = (P V) W_out + b_out

Sharding: 8 cores = (4 batches) x (2 query-halves). Each core gets its
batch's x rotated so its 1024 query rows come first; keys are the full
2048 rows (key order is irrelevant to the result). K/V projections are
duplicated between the 2 cores of a batch; no collectives needed.

Design (modeled 246.7us vs 395.7us baseline):
  - Host prepacks x^T and W_qkv tiles as fp8(e4m3) hi/lo residual pairs
    (W scaled by 32 so residuals stay in e4m3 normal range; undone via
    the exp scale 2^-13 and recip*2^-5), and W_out in bf16 — all in the
    exact SBUF layouts, so the device program is a pure matmul pipeline:
    no transposes, no dtype conversions, no DRAM staging round-trips.
  - Projections AND scores run as fp8 DoubleRow matmuls (K=256/instr at
    0.5 cyc/row): hi*hi + hi*lo + lo*hi keeps ~bf16 accuracy at 2/3 the
    PE time. K^T/Q^T are stored as fp8 hi/lo pairs produced during the
    projection PSUM drains (copy + subtract on DVE/ACT). PV/out stay
    bf16 (1 cyc/row, fp32 PSUM).
  - Everything SBUF-resident: K^T, V, Q^T, P, O^T live in SBUF between
    phases; only inputs in / y out cross HBM (one in-order DMA queue in
    first-use order; small first chunks so PE starts at ~4us).
  - Softmax skips the max-subtraction (scores ~ N(0,4): exp stays well
    inside fp32 range); row sums accumulate across all key tiles in one
    PSUM region via tiny ones-matmuls; normalization by 1/rowsum is
    deferred to the final output projection.
  - PSUM pools are laid out on the two allocator stacks so consecutive
    phases coexist (no alloc-on-release stalls): left [pjp(3)] ->
    opp(3); right [sumsp(1), sps(4)] -> ypp(4). Zero PE gaps after the
    DMA-bound start; p-state stays hot end to end.
"""

import sys

sys.path.insert(0, "/opt/trn_rl_repo")

import numpy as np

import concourse.bass as bass
import concourse.mybir as mybir
import concourse.tile as tile
from concourse import bacc

B, N, E = 4, 2048, 1024
NQ = N // 2          # query rows per core
P = 128              # partitions
FT = E // P          # 8 feature tiles (contraction for projections)
ET = E // P          # 8 embed tiles
MT = N // P          # 16 key tiles
QT = NQ // P         # 8 query tiles
F32 = mybir.dt.float32
BF16 = mybir.dt.bfloat16
FP8 = mybir.dt.float8e4
DR = mybir.MatmulPerfMode.DoubleRow


def build_program():
    nc = bacc.Bacc("TRN2", target_bir_lowering=False, debug=False)
    # Host-prepacked bf16 operands, already in SBUF tile layouts:
    #   xT[p, f, n]  = x_rot[n, f*128+p]
    #   wkb[p, f, e] = Wk[f*128+p, e]   (same for wqb / wvb)
    #   wob[p, e, c] = W_out[e*128+p, c]
    xh_d = nc.dram_tensor("xh", [P, FT, N], FP8, kind="ExternalInput").ap()
    xl_d = nc.dram_tensor("xl", [P, FT, N], FP8, kind="ExternalInput").ap()
    w8_d = {}
    for nm in ("wkh", "wkl", "wqh", "wql", "wvh", "wvl"):
        w8_d[nm] = nc.dram_tensor(nm, [P, FT, E], FP8,
                                  kind="ExternalInput").ap()
    wob_d = nc.dram_tensor("wob", [P, ET, E], BF16, kind="ExternalInput").ap()
    bout = nc.dram_tensor("bout", [E], F32, kind="ExternalInput").ap()
    y = nc.dram_tensor("y", [NQ, E], F32, kind="ExternalOutput").ap()

    with tile.TileContext(nc) as tc:
        _body(nc, tc, xh_d, xl_d, w8_d, wob_d, bout, y)
    nc.compile()
    return nc


def _body(nc, tc, xh_d, xl_d, w8_d, wob_d, bout, y):
    KB = 4           # key blocks of 512 for projections
    KW = N // KB     # 512 key cols per block
    QF = 2           # f-quarter chunk width for the early DMAs

    # Persistent SBUF pools first (pools are LIFO stacks per side;
    # long-lived pools must sit at the bottom).
    kTp = tc.alloc_tile_pool(name="kTp", bufs=1)
    qTp = tc.alloc_tile_pool(name="qTp", bufs=1)
    vp = tc.alloc_tile_pool(name="vp", bufs=1, side="right")
    kTh = kTp.tile([P, ET, N], FP8, name="kTh", tag="kTh")
    kTl = kTp.tile([P, ET, N], FP8, name="kTl", tag="kTl")
    qTh = qTp.tile([P, ET, NQ], FP8, name="qTh", tag="qTh")
    qTl = qTp.tile([P, ET, NQ], FP8, name="qTl", tag="qTl")
    v = vp.tile([P, MT, E], BF16, name="v", tag="v")

    # PSUM pools, two-sided so phases coexist without alloc-on-release
    # stalls: left [sumsp(1 bank), pjp(3 banks)], right [sps(4 banks)].
    # Later: pop pjp+sumsp -> opp(4, left); pop sps -> ypp(4, right).
    pjp = tc.alloc_tile_pool(name="pjp", bufs=1, space="PSUM")
    sumsp = tc.alloc_tile_pool(name="sumsp", bufs=1, space="PSUM",
                               side="right")
    sps = tc.alloc_tile_pool(name="sps", bufs=2, space="PSUM", side="right")
    sums_ps = sumsp.tile([P, QT], F32, name="sums_ps", tag="sums_ps")

    # ---- Phase A pools: projections (released before scores/PV) ----
    xTp = tc.alloc_tile_pool(name="xTp", bufs=1)
    wp = tc.alloc_tile_pool(name="wp", bufs=1)
    xh = xTp.tile([P, FT, N], FP8, name="xh", tag="xh")
    xl = xTp.tile([P, FT, N], FP8, name="xl", tag="xl")
    w8 = {nm: wp.tile([P, FT, E], FP8, name=nm, tag=nm)
          for nm in ("wkh", "wkl", "wqh", "wql", "wvh", "wvl")}

    # One in-order DMA queue in first-use order (DMA engines are an
    # exclusive resource in practice; a second queue just lets low-
    # priority transfers jump ahead of critical ones).
    for qtr in range(4):
        f0, f1 = qtr * QF, (qtr + 1) * QF
        nc.sync.dma_start(out=xh[:, f0:f1, 0:KW], in_=xh_d[:, f0:f1, 0:KW])
        nc.sync.dma_start(out=w8["wkh"][:, f0:f1, :],
                          in_=w8_d["wkh"][:, f0:f1, :])
        nc.sync.dma_start(out=xl[:, f0:f1, 0:KW], in_=xl_d[:, f0:f1, 0:KW])
        nc.sync.dma_start(out=w8["wkl"][:, f0:f1, :],
                          in_=w8_d["wkl"][:, f0:f1, :])
    for blk in range(1, KB):
        nc.sync.dma_start(out=xh[:, :, blk * KW:(blk + 1) * KW],
                          in_=xh_d[:, :, blk * KW:(blk + 1) * KW])
        nc.sync.dma_start(out=xl[:, :, blk * KW:(blk + 1) * KW],
                          in_=xl_d[:, :, blk * KW:(blk + 1) * KW])
    for nm in ("wqh", "wql", "wvh", "wvl"):
        nc.sync.dma_start(out=w8[nm], in_=w8_d[nm])

    # (whi, xhi) + (whi, xlo) + (wlo, xhi) DoubleRow product terms per
    # f-pair: 12 fp8 matmuls at 0.5 cyc/row replace 8 bf16 matmuls.
    PROD = ((0, 0), (0, 1), (1, 0))

    def dr_proj(ps, whi, wlo, ecols, xcols, j, first, last):
        fsl = slice(2 * j, 2 * j + 2)
        for t, (wi, xi) in enumerate(PROD):
            wt = whi if wi == 0 else wlo
            xt = xh if xi == 0 else xl
            nc.tensor.matmul(ps, wt[:, fsl, ecols], xt[:, fsl, xcols],
                             start=(first and t == 0), stop=(last and t == 2),
                             perf_mode=DR)

    def dr_v(ps, wvcols, mcols, j, first, last):
        fsl = slice(2 * j, 2 * j + 2)
        for t, (xi, wi) in enumerate(PROD):
            xt = xh if xi == 0 else xl
            wt = w8["wvh"] if wi == 0 else w8["wvl"]
            nc.tensor.matmul(ps, xt[:, fsl, mcols], wt[:, fsl, wvcols],
                             start=(first and t == 0), stop=(last and t == 2),
                             perf_mode=DR)

    def pj_copy(dst, ps, i):
        if i % 2 == 0:
            nc.vector.tensor_copy(dst, ps)
        else:
            nc.scalar.activation(dst, ps, mybir.ActivationFunctionType.Copy)

    def pj_copy8(dst_h, dst_l, ps, i):
        if i % 2 == 0:
            nc.vector.tensor_copy(dst_h, ps)
            nc.vector.tensor_tensor(out=dst_l, in0=ps, in1=dst_h,
                                    op=mybir.AluOpType.subtract)
        else:
            nc.scalar.activation(dst_h, ps, mybir.ActivationFunctionType.Copy)
            nc.vector.tensor_tensor(out=dst_l, in0=ps, in1=dst_h,
                                    op=mybir.AluOpType.subtract)

    # K projection: kT[:, e, kcols] = sum_f wk[:, f, e*]^T xT[:, f, kcols]
    # Block 0 runs in waves of 3 e-tiles, f-quarter-major, so the first
    # matmuls fire as soon as the first small DMA chunks land.
    for wave in ((0, 1, 2), (3, 4, 5), (6, 7)):
        ps0 = {e: pjp.tile([P, KW], F32, name=f"pjk0_{e}", tag=f"pj{e % 3}")
               for e in wave}
        for j in range(4):
            for e in wave:
                dr_proj(ps0[e], w8["wkh"], w8["wkl"],
                        slice(e * P, (e + 1) * P), slice(0, KW), j,
                        first=(j == 0), last=(j == 3))
        for e in wave:
            pj_copy8(kTh[:, e, 0:KW], kTl[:, e, 0:KW], ps0[e], e)
    for blk in range(1, KB):
        for e in range(ET):
            ps = pjp.tile([P, KW], F32, name="pjk", tag=f"pj{e % 3}")
            for j in range(4):
                dr_proj(ps, w8["wkh"], w8["wkl"], slice(e * P, (e + 1) * P),
                        slice(blk * KW, (blk + 1) * KW), j,
                        first=(j == 0), last=(j == 3))
            pj_copy8(kTh[:, e, blk * KW:(blk + 1) * KW],
                     kTl[:, e, blk * KW:(blk + 1) * KW], ps, e)

    # Q projection (queries = first NQ rows of rotated x)
    for blk in range(2):
        for e in range(ET):
            ps = pjp.tile([P, KW], F32, name="pjq", tag=f"pj{e % 3}")
            for j in range(4):
                dr_proj(ps, w8["wqh"], w8["wql"], slice(e * P, (e + 1) * P),
                        slice(blk * KW, (blk + 1) * KW), j,
                        first=(j == 0), last=(j == 3))
            pj_copy8(qTh[:, e, blk * KW:(blk + 1) * KW],
                     qTl[:, e, blk * KW:(blk + 1) * KW], ps, e)

    # V projection: v[:, m, :] rows = sum_f xT[:, f, m*]^T wv[:, f, :]
    for m in range(MT):
        for hh in range(2):
            ps = pjp.tile([P, E // 2], F32, name="pjv",
                          tag=f"pj{(2 * m + hh) % 3}")
            for j in range(4):
                dr_v(ps, slice(hh * (E // 2), (hh + 1) * (E // 2)),
                     slice(m * P, (m + 1) * P), j,
                     first=(j == 0), last=(j == 3))
            nsp = 2 if m == MT - 1 else 1
            for sp_i in range(nsp):
                w = (E // 2) // nsp
                c0 = hh * (E // 2) + sp_i * w
                pj_copy(v[:, m, c0:c0 + w], ps[:, sp_i * w:(sp_i + 1) * w],
                        hh + sp_i)

    wp.release()
    xTp.release()

    # ---- Phase B pools ----
    pp = tc.alloc_tile_pool(name="pp", bufs=1)
    oTp = tc.alloc_tile_pool(name="oTp", bufs=1, side="right")
    wop = tc.alloc_tile_pool(name="wop", bufs=1, side="right")
    smp = tc.alloc_tile_pool(name="smp", bufs=1, side="right")
    p_t = pp.tile([P, MT, NQ], BF16, name="p_t", tag="p_t")
    oT = oTp.tile([P, ET, NQ], BF16, name="oT", tag="oT")
    wo = wop.tile([P, ET, E], BF16, name="wo", tag="wo")
    bo_b = wop.tile([P, E], F32, name="bo_b", tag="bo_b")
    ones = smp.tile([P, 1], BF16, name="ones", tag="ones")
    sums = smp.tile([P, QT], F32, name="sums", tag="sums")
    recip = smp.tile([P, QT], F32, name="recip", tag="recip")

    nc.sync.dma_start(out=wo, in_=wob_d)
    bout_bcast = bass.AP(tensor=bout.tensor, offset=0, ap=[[0, P], [1, E]])
    nc.sync.dma_start(out=bo_b, in_=bout_bcast)
    nc.vector.memset(ones, 1.0)

    # Scores: s^T[k, q] per key tile; P = exp(s/8); rowsums via
    # ones-matmuls accumulated across all tiles in one PSUM region.
    for m in range(MT):
        s = sps.tile([P, NQ], F32, name="s", tag="s")
        for hh in range(2):
            qsl = slice(hh * (NQ // 2), (hh + 1) * (NQ // 2))
            msl = slice(m * P, (m + 1) * P)
            for j in range(4):
                esl = slice(2 * j, 2 * j + 2)
                for t, (ki, qi) in enumerate(PROD):
                    kt = kTh if ki == 0 else kTl
                    qt = qTh if qi == 0 else qTl
                    nc.tensor.matmul(
                        s[:, qsl], kt[:, esl, msl], qt[:, esl, qsl],
                        start=(j == 0 and t == 0),
                        stop=(j == 3 and t == 2), perf_mode=DR)
        for hh in range(2):
            nc.scalar.activation(
                p_t[:, m, hh * (NQ // 2):(hh + 1) * (NQ // 2)],
                s[:, hh * (NQ // 2):(hh + 1) * (NQ // 2)],
                mybir.ActivationFunctionType.Exp, scale=0.125 / 1024.0)
        # Row-sum the PREVIOUS tile's exp so PE never waits on ACT.
        if m > 0:
            _row_sums(nc, p_t, m - 1, ones, sums_ps, first=(m == 1),
                      last=False)

    pjp.release()

    # PV: oT[e, q] accumulated over all m; 16 (e, query-half) groups of
    # [128, 512] on the 3 banks freed by pjp (ring-3: copies of group i
    # drain while i+1/i+2 run). The h0 groups go first so only exp(15)
    # of the first query half gates the phase start; the final rowsums
    # batch hides behind the first PV group.
    H = NQ // 2
    opp = tc.alloc_tile_pool(name="opp", bufs=3, space="PSUM")
    first_pv = True
    for hh in range(2):
        for e in range(ET):
            o_ps = opp.tile([P, H], F32, name=f"o{e}_{hh}", tag="o")
            for m in range(MT):
                nc.tensor.matmul(
                    o_ps, v[:, m, e * P:(e + 1) * P],
                    p_t[:, m, hh * H:(hh + 1) * H],
                    start=(m == 0), stop=(m == MT - 1))
            if e % 2 == 0:
                nc.vector.tensor_copy(oT[:, e, hh * H:(hh + 1) * H], o_ps)
            else:
                nc.scalar.activation(oT[:, e, hh * H:(hh + 1) * H], o_ps,
                                     mybir.ActivationFunctionType.Copy)
            if first_pv:
                # Tiny matmuls + DVE work, hidden behind PV group 0.
                _row_sums(nc, p_t, MT - 1, ones, sums_ps, first=False,
                          last=True)
                nc.vector.tensor_copy(sums, sums_ps)
                nc.vector.reciprocal(recip, sums)
                nc.vector.tensor_scalar(out=recip, in0=recip,
                                        scalar1=1.0 / 32.0, scalar2=None,
                                        op0=mybir.AluOpType.mult)
                first_pv = False

    sps.release()

    # Output projection: y rows = (O_u W_out) * recip + b_out
    ypp = tc.alloc_tile_pool(name="ypp", bufs=2, space="PSUM", side="right")
    with tc.tile_pool(name="ysb", bufs=3) as ysp:
        HE = E // 2
        for nqt in range(QT):
            yps = ypp.tile([P, E], F32, name="yps", tag="yps")
            for e in range(ET):
                for hh in range(2):
                    nc.tensor.matmul(
                        yps[:, hh * HE:(hh + 1) * HE],
                        oT[:, e, nqt * P:(nqt + 1) * P],
                        wo[:, e, hh * HE:(hh + 1) * HE],
                        start=(e == 0), stop=(e == ET - 1))
            ysb = ysp.tile([P, E], F32, name="ysb", tag="ysb")
            # Split the last tile's finalize chain to shorten the kernel
            # tail (ACT scale -> DVE bias-add -> DMA out).
            nsp = 2 if nqt == QT - 1 else 1
            w = E // nsp
            for sp_i in range(nsp):
                c0 = sp_i * w
                nc.scalar.activation(ysb[:, c0:c0 + w], yps[:, c0:c0 + w],
                                     mybir.ActivationFunctionType.Copy,
                                     scale=recip[:, nqt:nqt + 1])
                nc.vector.tensor_tensor(out=ysb[:, c0:c0 + w],
                                        in0=ysb[:, c0:c0 + w],
                                        in1=bo_b[:, c0:c0 + w],
                                        op=mybir.AluOpType.add)
                nc.sync.dma_start(
                    out=y[nqt * P:(nqt + 1) * P, c0:c0 + w],
                    in_=ysb[:, c0:c0 + w])

    ypp.release()
    sumsp.release()
    opp.release()
    smp.release()
    wop.release()
    oTp.release()
    pp.release()
    qTp.release()
    kTp.release()
    vp.release()


def _row_sums(nc, p_t, m, ones, sums_ps, first, last):
    for q in range(QT):
        nc.tensor.matmul(sums_ps[:, q:q + 1],
                         p_t[:, m, q * P:(q + 1) * P], ones,
                         start=(first and q == 0),
                         stop=(last and q == QT - 1),
                         skip_group_check=True)


_NC_CACHE = None


def _get_program():
    global _NC_CACHE
    if _NC_CACHE is None:
        _NC_CACHE = build_program()
    return _NC_CACHE


def _pack_w(w, dtype):
    # [E, C] -> [P, E//P, C] with rows f*128+p on partition p
    return np.ascontiguousarray(
        w.reshape(FT, P, -1).transpose(1, 0, 2)).astype(dtype)


def _split_fp8(a):
    import ml_dtypes
    f8 = ml_dtypes.float8_e4m3
    hi = a.astype(f8)
    lo = (a - hi.astype(np.float32)).astype(f8)
    return hi, lo


def kernel(x, W_qkv, W_out, b_out):
    import ml_dtypes
    from concourse.bass_utils import run_bass_kernel_spmd

    bf16 = ml_dtypes.bfloat16
    x = np.asarray(x, dtype=np.float32)
    W_qkv = np.asarray(W_qkv, dtype=np.float32)
    W_out = np.asarray(W_out, dtype=np.float32)
    b_out = np.asarray(b_out, dtype=np.float32)

    Wq32 = W_qkv * np.float32(32.0)  # exact pow2; undone via exp/recip
    wkh, wkl = _split_fp8(_pack_w(Wq32[:, E:2 * E], np.float32))
    wqh, wql = _split_fp8(_pack_w(Wq32[:, 0:E], np.float32))
    wvh, wvl = _split_fp8(_pack_w(Wq32[:, 2 * E:], np.float32))
    wob = _pack_w(W_out, bf16)

    nc = _get_program()
    in_maps = []
    for c in range(8):
        b, half = divmod(c, 2)
        xb = x[b]
        xrot = np.concatenate([xb[half * NQ:], xb[:half * NQ]], axis=0)
        # xT[p, f, n] = xrot[n, f*128+p]
        xT = np.ascontiguousarray(
            xrot.T.reshape(FT, P, N).transpose(1, 0, 2)).astype(np.float32)
        xhp, xlp = _split_fp8(xT)
        in_maps.append({"xh": xhp, "xl": xlp, "wkh": wkh, "wkl": wkl,
                        "wqh": wqh, "wql": wql, "wvh": wvh, "wvl": wvl,
                        "wob": wob, "bout": b_out})
    res = run_bass_kernel_spmd(nc, in_maps, list(range(8)))
    out = np.empty((B, N, E), dtype=np.float32)
    for c in range(8):
        b, half = divmod(c, 2)
        out[b, half * NQ:(half + 1) * NQ] = res.results[c]["y"]
    return out
